# revision 1
# baseline (speedup 1.0000x reference)
"""Self-contained Trainium2 kernel for the dense transformer block
(B=4, T=2048, C=1024, H=16, MLP 4x, hybrid cond/causal mask), SPMD over
8 NeuronCores, collective-free. See build_nc for the device program."""
import sys
sys.path.insert(0, '/opt/trn_rl_repo')
import numpy as np
import ml_dtypes
import concourse.bass as bass
import concourse.mybir as mybir
import concourse.tile as tile
from concourse.vector_clock import ScopedClock
from concourse.bass_utils import run_bass_kernel_spmd

# ---- walrus 1-sync-wait-per-instruction workarounds ----
import concourse.bass as bass
import concourse.mybir as mybir
import concourse.tile as tile
from concourse.vector_clock import ScopedClock

_installed = False


def _split_multi_waits(ordered_by_block, nc):
    for bb_name, insts in ordered_by_block.items():
        need = any(
            inst.sync_info is not None and len(inst.sync_info.on_wait) > 1
            for inst in insts
        )
        if not need:
            continue
        new_list = []
        for inst in insts:
            si = inst.sync_info
            waits = list(si.on_wait) if si is not None and si.on_wait else []
            if len(waits) > 1:
                for w in waits[:-1]:
                    nop = mybir.InstNoOp(
                        name=nc.get_next_instruction_name(),
                        ins=[],
                        outs=[],
                        bass_nofuse=True,
                    )
                    nop.engine = inst.engine
                    nop.sync_info = mybir.SyncInfo(on_wait=[w], on_update=[])
                    new_list.append(nop)
                ups = list(si.on_update) if si.on_update else []
                inst.sync_info = mybir.SyncInfo(on_wait=[waits[-1]], on_update=ups)
            new_list.append(inst)
        insts[:] = new_list


class _SplitWaitClockWait:
    def __init__(self, tc, ordered, **kw):
        import bass_rust
        self._inner = bass_rust.TileClockWait(tc, ordered, **kw)
        self._tc = tc
        self._ordered = ordered

    def __getattr__(self, a):
        return getattr(self._inner, a)

    def assign_waits(self, bb_name):
        r = self._inner.assign_waits(bb_name)
        _split_multi_waits(self._ordered, self._tc.nc)
        return r


class PatchedTileContext(tile.TileContext):
    """TileContext whose final drain carries at most one sem wait."""

    def _drain_and_barrier(self, tick_clock, wait_clock):
        probe = self.nc.sync.nop(nofuse=True)
        add = wait_clock.add_sem_waits
        add(probe.ins, ScopedClock({None: tick_clock.global_clock}))
        si = probe.ins.sync_info
        waits = list(si.on_wait) if si is not None and si.on_wait else []
        if len(waits) > 1:
            probe.ins.sync_info = mybir.SyncInfo(on_wait=[waits[0]], on_update=[])
            for w in waits[1:]:
                n = self.nc.sync.nop(nofuse=True)
                n.ins.sync_info = mybir.SyncInfo(on_wait=[w], on_update=[])
        self.nc.sync.drain()
        self.nc.all_engine_barrier()
        popped = self.nc._tile_sem_poison_stack.pop()
        assert popped is self._sem_poison
        self.nc.clear_and_free_semaphores(list(self.sems.allocated().values()))
        self.nc.all_engine_barrier()


def _install():
    global _installed
    if not _installed:
        tile.TileClockWait = _SplitWaitClockWait
        _installed = True

_install()


# ---- kernel ----

F32 = mybir.dt.float32
BF16 = mybir.dt.bfloat16
AF = mybir.ActivationFunctionType
ALU = mybir.AluOpType

C = 1024
T = 2048
H = 16
HD = 64
FF = 4096
COND = 256
EPS = 1e-5
NJ = 4
QB = 256
NCHUNK = 16
SCALE = 1.0 / np.sqrt(HD)

TUNE = {"xin": 6, "cast": 2, "wsl": 4, "wv8": 2, "se": 4, "kq": 2, "sml": 1, "bc": 1}


def kv_limit(j):
    return 4 * j + 4


def mask_slots():
    slots = []
    for j in range(NJ):
        for m in ([2, 3] if j == 0 else [4 * j, 4 * j + 1, 4 * j + 2, 4 * j + 3]):
            slots.append((j, m))
    return slots


MASK_SLOTS = mask_slots()
MASK_IDX = {s: i for i, s in enumerate(MASK_SLOTS)}
NMASK = len(MASK_SLOTS)


def build_nc(debug=False):
    nc = bass.Bass()
    xT = nc.dram_tensor("xT", [C, T], F32, kind="ExternalInput")
    xqT = nc.dram_tensor("xqT", [C, 1024], F32, kind="ExternalInput")
    wq = nc.dram_tensor("wq", [C, C], BF16, kind="ExternalInput")
    wk = nc.dram_tensor("wk", [C, C], BF16, kind="ExternalInput")
    wv = nc.dram_tensor("wv", [C, C], BF16, kind="ExternalInput")
    wp = nc.dram_tensor("wp", [C, C], BF16, kind="ExternalInput")
    w1 = nc.dram_tensor("w1", [C, FF], BF16, kind="ExternalInput")
    w2 = nc.dram_tensor("w2", [FF, C], BF16, kind="ExternalInput")
    bq = nc.dram_tensor("bq", [C], F32, kind="ExternalInput")
    bvb = nc.dram_tensor("bvb", [128, C], BF16, kind="ExternalInput")
    bp = nc.dram_tensor("bp", [C], F32, kind="ExternalInput")
    b1 = nc.dram_tensor("b1", [FF], F32, kind="ExternalInput")
    b2 = nc.dram_tensor("b2", [C], F32, kind="ExternalInput")
    g1 = nc.dram_tensor("g1", [C], F32, kind="ExternalInput")
    bt1 = nc.dram_tensor("bt1", [C], F32, kind="ExternalInput")
    g2 = nc.dram_tensor("g2", [C], F32, kind="ExternalInput")
    bt2 = nc.dram_tensor("bt2", [C], F32, kind="ExternalInput")
    masks = nc.dram_tensor("masks", [NMASK * 128, QB], BF16, kind="ExternalInput")
    outT = nc.dram_tensor("outT", [C, 1024], F32, kind="ExternalOutput")

    stats1 = nc.dram_tensor("stats1", [2, T], F32)
    statsq = nc.dram_tensor("statsq", [2, 1024], F32)
    stats2 = nc.dram_tensor("stats2", [2, 1024], F32)
    x2q_d = nc.dram_tensor("x2q_d", [C, 1024], F32)

    dbg = {}
    if debug:
        dbg["hT"] = nc.dram_tensor("dbg_hT", [C, T], F32, kind="ExternalOutput")
        dbg["v"] = nc.dram_tensor("dbg_v", [T, H * HD], F32, kind="ExternalOutput")
        dbg["yT"] = nc.dram_tensor("dbg_yT", [C, 1024], F32, kind="ExternalOutput")
        dbg["x2T"] = nc.dram_tensor("dbg_x2T", [C, 1024], F32, kind="ExternalOutput")

    with PatchedTileContext(nc) as tc:
        with (
            tc.tile_pool(name="big", bufs=1) as big,
            tc.tile_pool(name="kq", bufs=TUNE["kq"]) as kq_p,
            tc.tile_pool(name="st", bufs=1) as st,
            tc.tile_pool(name="xin", bufs=TUNE["xin"]) as xin_p,
            tc.tile_pool(name="cast", bufs=TUNE["cast"]) as cast_p,
            tc.tile_pool(name="wsl", bufs=TUNE["wsl"]) as wsl_p,
            tc.tile_pool(name="wv8", bufs=TUNE["wv8"]) as wv8_p,
            tc.tile_pool(name="bc", bufs=TUNE["bc"]) as bc_p,
            tc.tile_pool(name="se", bufs=TUNE["se"]) as se_p,
            tc.tile_pool(name="rows2", bufs=2) as rows2_p,
            tc.tile_pool(name="rows1", bufs=1) as rows1_p,
            tc.tile_pool(name="sml", bufs=TUNE["sml"]) as sml_p,
            tc.tile_pool(name="outp", bufs=2) as out_p,
            tc.tile_pool(name="psb", bufs=2, space="PSUM") as psb,
            tc.tile_pool(name="pss", bufs=4, space="PSUM") as pss,
        ):
            hT = big.tile([128, 8, T], BF16, tag="A")
            hqT = big.tile([128, 8, 1024], BF16, tag="E")
            yT = big.tile([128, 8, 1024], BF16, tag="Y")
            v_ext = big.tile([128, NCHUNK, H, HD + 1], BF16, tag="B")
            mask_sb = st.tile([128, NMASK, QB], BF16)
            bvb_sb = st.tile([128, C], BF16)
            ones1 = st.tile([128, 1], BF16)
            ones64 = st.tile([1, 64], BF16)

            eps1 = st.tile([1, 1], F32)
            nc.vector.memset(eps1, EPS)
            nc.vector.memset(ones1, 1.0)
            nc.vector.memset(ones64, 1.0)
            nc.sync.dma_start(out=bvb_sb, in_=bvb[:, :])
            nc.sync.dma_start(out=mask_sb, in_=masks.rearrange("(s p) q -> p s q", p=128))

            def load_cvec(name, dram, n=8):
                ts = []
                for cc in range(n):
                    t = st.tile([128, 1], F32, tag=f"{name}{cc}")
                    nc.sync.dma_start(out=t, in_=dram[128 * cc:128 * (cc + 1)][:, None])
                    ts.append(t)
                return ts

            bq_sb = load_cvec("bq", bq)
            bp_sb = load_cvec("bp", bp)
            b2_sb = load_cvec("b2", b2)
            b1_sb = load_cvec("b1", b1, n=32)
            g1_sb = load_cvec("g1", g1)
            bt1_sb = load_cvec("bt1", bt1)
            g2_sb = load_cvec("g2", g2)
            bt2_sb = load_cvec("bt2", bt2)

            # ---------- transposed layernorm (dram -> sbuf bf16) ----------
            def ln_T(x_dram, W, stats_dram, out_tile, g_sb, bt_sb):
                nt = W // 512
                for t in range(nt):
                    mu_ps = pss.tile([1, 512], F32, tag="pstd")
                    sq_ps = pss.tile([1, 512], F32, tag="pstd")
                    for cc in range(8):
                        xt = xin_p.tile([128, 512], F32, tag="x")
                        nc.sync.dma_start(
                            out=xt, in_=x_dram[128 * cc:128 * (cc + 1), 512 * t:512 * (t + 1)])
                        xb = cast_p.tile([128, 512], BF16, tag="xb")
                        nc.vector.tensor_copy(out=xb, in_=xt)
                        sq = cast_p.tile([128, 512], BF16, tag="sq")
                        nc.scalar.activation(out=sq, in_=xt, func=AF.Square)
                        nc.tensor.matmul(mu_ps, ones1, xb, start=(cc == 0), stop=(cc == 7))
                        nc.tensor.matmul(sq_ps, ones1, sq, start=(cc == 0), stop=(cc == 7))
                    mu = rows2_p.tile([1, 512], F32, tag="mu")
                    ex2 = rows2_p.tile([1, 512], F32, tag="ex2")
                    nc.vector.tensor_scalar_mul(mu, mu_ps, 1.0 / C)
                    nc.vector.tensor_scalar_mul(ex2, sq_ps, 1.0 / C)
                    var = rows1_p.tile([1, 512], F32, tag="var")
                    nc.vector.scalar_tensor_tensor(
                        out=var, in0=mu, scalar=-1.0, in1=mu, op0=ALU.mult, op1=ALU.mult)
                    nc.vector.tensor_add(var, var, ex2)
                    std = rows1_p.tile([1, 512], F32, tag="std")
                    nc.scalar.activation(out=std, in_=var, func=AF.Sqrt, bias=eps1)
                    rstd = rows1_p.tile([1, 512], F32, tag="rstd")
                    nc.vector.reciprocal(out=rstd, in_=std)
                    nc.sync.dma_start(out=stats_dram[0, 512 * t:512 * (t + 1)][None, :], in_=mu)
                    nc.sync.dma_start(out=stats_dram[1, 512 * t:512 * (t + 1)][None, :], in_=rstd)
                for t in range(nt):
                    mu_b = bc_p.tile([128, 512], F32, tag="mu_b")
                    rstd_b = bc_p.tile([128, 512], F32, tag="rstd_b")
                    nc.sync.dma_start(out=mu_b, in_=bass.AP(
                        tensor=stats_dram[0][None, :].tensor,
                        offset=512 * t, ap=[[0, 128], [1, 512]]))
                    nc.sync.dma_start(out=rstd_b, in_=bass.AP(
                        tensor=stats_dram[1][None, :].tensor,
                        offset=W + 512 * t, ap=[[0, 128], [1, 512]]))
                    for cc in range(8):
                        xt = xin_p.tile([128, 512], F32, tag="x")
                        nc.sync.dma_start(
                            out=xt, in_=x_dram[128 * cc:128 * (cc + 1), 512 * t:512 * (t + 1)])
                        d = cast_p.tile([128, 512], F32, tag="diff")
                        nc.vector.tensor_sub(d, xt, mu_b)
                        hp = cast_p.tile([128, 512], F32, tag="hpart")
                        nc.vector.scalar_tensor_tensor(
                            out=hp, in0=d, scalar=g_sb[cc], in1=rstd_b,
                            op0=ALU.mult, op1=ALU.mult)
                        nc.vector.tensor_scalar_add(
                            out_tile[:, cc, 512 * t:512 * (t + 1)], hp, bt_sb[cc])

            ln_T(xT, T, stats1, hT, g1_sb, bt1_sb)
            ln_T(xqT, 1024, statsq, hqT, g1_sb, bt1_sb)

            if debug:
                for cc in range(8):
                    for t in range(4):
                        dt_ = out_p.tile([128, 512], F32, tag="dbg")
                        nc.vector.tensor_copy(out=dt_, in_=hT[:, cc, 512 * t:512 * (t + 1)])
                        nc.sync.dma_start(
                            out=dbg["hT"][128 * cc:128 * (cc + 1), 512 * t:512 * (t + 1)],
                            in_=dt_)

            # ---------- V (normal layout, +bv, ones col) ----------
            for fh in range(4):
                wv_sl = wv8_p.tile([128, 8, 256], BF16, tag="wv")
                nc.sync.dma_start(
                    out=wv_sl,
                    in_=wv[:, 256 * fh:256 * (fh + 1)].rearrange("(c p) f -> p c f", p=128))
                for m in range(NCHUNK):
                    vps = pss.tile([128, 256], F32, tag="pstd")
                    for cc in range(8):
                        nc.tensor.matmul(
                            vps, hT[:, cc, 128 * m:128 * (m + 1)], wv_sl[:, cc, :],
                            start=(cc == 0), stop=(cc == 7))
                    nc.vector.tensor_add(
                        v_ext[:, m, 4 * fh:4 * (fh + 1), 0:HD],
                        vps.rearrange("p (h d) -> p h d", h=4),
                        bvb_sb[:, 256 * fh:256 * (fh + 1)].rearrange("p (h d) -> p h d", h=4))
            for m in range(NCHUNK):
                nc.vector.memset(v_ext[:, m, :, HD:HD + 1], 1.0)

            if debug:
                for m in range(NCHUNK):
                    dt_ = out_p.tile([128, H * HD], F32, tag="dbgv")
                    nc.vector.tensor_copy(
                        out=dt_.rearrange("p (h d) -> p h d", h=H), in_=v_ext[:, m, :, 0:HD])
                    nc.sync.dma_start(out=dbg["v"][128 * m:128 * (m + 1), :], in_=dt_)

            # ---------- attention (lazy K/Q per head pair) ----------
            for p in range(8):
                kpair = kq_p.tile([128, T], BF16, tag="KP")
                wk_sl = wsl_p.tile([128, 8, 128], BF16, tag="wk")
                nc.sync.dma_start(
                    out=wk_sl,
                    in_=wk[:, 128 * p:128 * (p + 1)].rearrange("(c p) f -> p c f", p=128))
                for t in range(4):
                    kps = pss.tile([128, 512], F32, tag="pstd")
                    for cc in range(8):
                        nc.tensor.matmul(
                            kps, wk_sl[:, cc, :], hT[:, cc, 512 * t:512 * (t + 1)],
                            start=(cc == 0), stop=(cc == 7))
                    nc.vector.tensor_copy(out=kpair[:, 512 * t:512 * (t + 1)], in_=kps)

                qpair = kq_p.tile([128, 1024], BF16, tag="QP")
                wq_sl = wsl_p.tile([128, 8, 128], BF16, tag="wk")
                nc.sync.dma_start(
                    out=wq_sl,
                    in_=wq[:, 128 * p:128 * (p + 1)].rearrange("(c p) f -> p c f", p=128))
                for t in range(2):
                    qps = pss.tile([128, 512], F32, tag="pstd")
                    for cc in range(8):
                        nc.tensor.matmul(
                            qps, wq_sl[:, cc, :], hqT[:, cc, 512 * t:512 * (t + 1)],
                            start=(cc == 0), stop=(cc == 7))
                    nc.vector.tensor_scalar_add(
                        qpair[:, 512 * t:512 * (t + 1)], qps, bq_sb[p])

                kstage = kq_p.tile([64, T], BF16, tag="KS")
                qstage = kq_p.tile([64, 1024], BF16, tag="QS")
                nc.sync.dma_start(out=kstage, in_=kpair[64:128, :])
                nc.sync.dma_start(out=qstage, in_=qpair[64:128, :])

                for odd in range(2):
                    h = 2 * p + odd
                    for j in range(NJ):
                        L = kv_limit(j)
                        qs = slice(QB * j, QB * (j + 1))
                        yps = pss.tile([65, QB], F32, tag="pstd")
                        se_ts = {}
                        for g0 in range(0, L, 4):
                            gl = min(4, L - g0)
                            sps = psb.tile([128, 4, QB], F32, tag="pbig")
                            for mi in range(gl):
                                m = g0 + mi
                                lhs = (kstage[:, 128 * m:128 * (m + 1)] if odd
                                       else kpair[0:64, 128 * m:128 * (m + 1)])
                                rhs = (qstage[:, qs] if odd else qpair[0:64, qs])
                                nc.tensor.matmul(
                                    sps[:, mi, :], lhs, rhs, start=True, stop=True)
                            se_t = se_p.tile([128, 4, QB], BF16, tag="se")
                            se_ts[g0] = se_t
                            nc.scalar.activation(
                                out=se_t[:, 0:gl, :], in_=sps[:, 0:gl, :],
                                func=AF.Exp, scale=float(SCALE))
                            for mi in range(gl):
                                m = g0 + mi
                                if (j, m) in MASK_IDX:
                                    kmi = MASK_IDX[(j, m)]
                                    nc.vector.tensor_mul(
                                        se_t[:, mi, :], se_t[:, mi, :], mask_sb[:, kmi, :])
                            for mi in range(gl):
                                m = g0 + mi
                                nc.tensor.matmul(
                                    yps, v_ext[:, m, h, :], se_t[:, mi, :],
                                    start=(m == 0), stop=(m == L - 1))
                        # normalize via reciprocal row sums
                        srow = sml_p.tile([65, QB], F32, tag="srow")
                        nc.vector.tensor_copy(out=srow[64:65, :], in_=yps[64:65, :])
                        sums0 = sml_p.tile([1, QB], F32, tag="sums0")
                        nc.sync.dma_start(out=sums0, in_=srow[64:65, :])
                        rsum = sml_p.tile([1, QB], F32, tag="rsum")
                        nc.vector.reciprocal(out=rsum, in_=sums0)
                        rsumb = sml_p.tile([1, QB], BF16, tag="rsumb")
                        nc.vector.tensor_copy(out=rsumb, in_=rsum)
                        rbps = pss.tile([64, QB], F32, tag="pstd")
                        nc.tensor.matmul(rbps, ones64, rsumb, start=True, stop=True)
                        rb = sml_p.tile([64, QB], BF16, tag="rb")
                        nc.vector.tensor_copy(out=rb, in_=rbps)
                        if odd:
                            yst = sml_p.tile([64, QB], BF16, tag="yst")
                            nc.vector.tensor_mul(yst, yps[0:64, :], rb)
                            nc.sync.dma_start(out=yT[64:128, p, qs], in_=yst)
                        else:
                            nc.vector.tensor_mul(yT[0:64, p, qs], yps[0:64, :], rb)

            if debug:
                for p in range(8):
                    for t in range(2):
                        dt_ = out_p.tile([128, 512], F32, tag="dbg")
                        nc.vector.tensor_copy(out=dt_, in_=yT[:, p, 512 * t:512 * (t + 1)])
                        nc.sync.dma_start(
                            out=dbg["yT"][128 * p:128 * (p + 1), 512 * t:512 * (t + 1)],
                            in_=dt_)

            # ---------- proj + residual -> x2q (dram) ----------
            for co in range(8):
                wp_sl = wsl_p.tile([128, 8, 128], BF16, tag="wk")
                nc.sync.dma_start(
                    out=wp_sl,
                    in_=wp[:, 128 * co:128 * (co + 1)].rearrange("(c p) f -> p c f", p=128))
                for t in range(2):
                    pps = pss.tile([128, 512], F32, tag="pstd")
                    for k in range(8):
                        nc.tensor.matmul(
                            pps, wp_sl[:, k, :], yT[:, k, 512 * t:512 * (t + 1)],
                            start=(k == 0), stop=(k == 7))
                    xt = xin_p.tile([128, 512], F32, tag="x")
                    nc.sync.dma_start(
                        out=xt, in_=xqT[128 * co:128 * (co + 1), 512 * t:512 * (t + 1)])
                    s_ = cast_p.tile([128, 512], F32, tag="diff")
                    nc.vector.tensor_add(s_, pps, xt)
                    x2t = cast_p.tile([128, 512], F32, tag="hpart")
                    nc.vector.tensor_scalar_add(x2t, s_, bp_sb[co])
                    nc.sync.dma_start(
                        out=x2q_d[128 * co:128 * (co + 1), 512 * t:512 * (t + 1)], in_=x2t)
                    if debug:
                        nc.sync.dma_start(
                            out=dbg["x2T"][128 * co:128 * (co + 1), 512 * t:512 * (t + 1)],
                            in_=x2t)

            # ---------- LN2 ----------
            h2T = big.tile([128, 8, 1024], BF16, tag="E")  # reuses hqT slot
            ln_T(x2q_d, 1024, stats2, h2T, g2_sb, bt2_sb)

            # ---------- MLP ----------
            gT = big.tile([128, 32, 512], BF16, tag="A")  # reuses hT slot
            for t in range(2):
                rs = slice(512 * t, 512 * (t + 1))
                for f in range(32):
                    w1_sl = wsl_p.tile([128, 8, 128], BF16, tag="wk")
                    nc.sync.dma_start(
                        out=w1_sl,
                        in_=w1[:, 128 * f:128 * (f + 1)].rearrange("(c p) f -> p c f", p=128))
                    fps = pss.tile([128, 512], F32, tag="pstd")
                    for cc in range(8):
                        nc.tensor.matmul(
                            fps, w1_sl[:, cc, :], h2T[:, cc, rs],
                            start=(cc == 0), stop=(cc == 7))
                    nc.scalar.activation(out=gT[:, f, :], in_=fps, func=AF.Gelu, bias=b1_sb[f])
                for co in range(8):
                    ops_ = pss.tile([128, 512], F32, tag="pstd")
                    for fq in range(4):
                        w2_sl = wsl_p.tile([128, 8, 128], BF16, tag="wk")
                        nc.sync.dma_start(
                            out=w2_sl,
                            in_=w2[:, 128 * co:128 * (co + 1)]
                            .rearrange("(fc p) f -> p fc f", p=128)[:, 8 * fq:8 * (fq + 1), :])
                        for fi in range(8):
                            f = 8 * fq + fi
                            nc.tensor.matmul(
                                ops_, w2_sl[:, fi, :], gT[:, f, :],
                                start=(f == 0), stop=(f == 31))
                    xt = xin_p.tile([128, 512], F32, tag="x")
                    nc.sync.dma_start(out=xt, in_=x2q_d[128 * co:128 * (co + 1), rs])
                    o1 = cast_p.tile([128, 512], F32, tag="diff")
                    nc.vector.tensor_add(o1, ops_, xt)
                    o2 = cast_p.tile([128, 512], F32, tag="hpart")
                    nc.vector.tensor_scalar_add(o2, o1, b2_sb[co])
                    nc.sync.dma_start(out=outT[128 * co:128 * (co + 1), rs], in_=o2)

    return nc


# ===================== host side =====================

def make_masks(rho):
    out = np.zeros((NMASK, 128, QB), dtype=np.float32)
    for i, (j, m) in enumerate(MASK_SLOTS):
        q0 = 512 * j + QB * rho
        q = q0 + np.arange(QB)[None, :]
        k = 128 * m + np.arange(128)[:, None]
        allowed = (k < COND) | ((q >= COND) & (k >= COND) & (k <= q))
        out[i] = allowed
    return out.reshape(NMASK * 128, QB).astype(ml_dtypes.bfloat16)


def shard_inputs(inputs):
    x = np.asarray(inputs["x"], np.float32)
    ins_common = {
        "wq": np.asarray(inputs["Wq"], np.float32).astype(ml_dtypes.bfloat16),
        "wk": np.asarray(inputs["Wk"], np.float32).astype(ml_dtypes.bfloat16),
        "wv": np.asarray(inputs["Wv"], np.float32).astype(ml_dtypes.bfloat16),
        "wp": np.asarray(inputs["Wp"], np.float32).astype(ml_dtypes.bfloat16),
        "w1": np.asarray(inputs["W1"], np.float32).astype(ml_dtypes.bfloat16),
        "w2": np.asarray(inputs["W2"], np.float32).astype(ml_dtypes.bfloat16),
        "bq": np.asarray(inputs["bq"], np.float32),
        "bvb": np.ascontiguousarray(np.broadcast_to(
            np.asarray(inputs["bv"], np.float32), (128, C))).astype(ml_dtypes.bfloat16),
        "bp": np.asarray(inputs["bp"], np.float32),
        "b1": np.asarray(inputs["b1"], np.float32),
        "b2": np.asarray(inputs["b2"], np.float32),
        "g1": np.asarray(inputs["ln1_g"], np.float32),
        "bt1": np.asarray(inputs["ln1_b"], np.float32),
        "g2": np.asarray(inputs["ln2_g"], np.float32),
        "bt2": np.asarray(inputs["ln2_b"], np.float32),
    }
    in_maps, row_sets = [], []
    for c in range(8):
        b, rho = c // 2, c % 2
        rows = np.concatenate(
            [np.arange(512 * j + QB * rho, 512 * j + QB * rho + QB) for j in range(4)])
        row_sets.append((b, rows))
        m = dict(ins_common)
        m["xT"] = np.ascontiguousarray(x[b].T)
        m["xqT"] = np.ascontiguousarray(x[b][rows].T)
        m["masks"] = make_masks(rho)
        in_maps.append(m)
    return in_maps, row_sets


_cached_nc = {}


def get_nc(debug=False):
    if debug not in _cached_nc:
        _cached_nc[debug] = build_nc(debug=debug)
    return _cached_nc[debug]


def run(inputs, debug=False):
    nc = get_nc(debug=debug)
    in_maps, row_sets = shard_inputs(inputs)
    res = run_bass_kernel_spmd(nc, in_maps, core_ids=list(range(8)))
    x = np.asarray(inputs["x"], np.float32)
    out = np.empty_like(x)
    for c in range(8):
        b, rows = row_sets[c]
        out[b][rows] = res.results[c]["outT"].T
    return out, res, row_sets


def kernel(**inputs):
    out, _, _ = run(inputs, debug=False)
    return out



# revision 16
# speedup vs baseline: 2.0843x; 2.0843x over previous
"""Self-contained Trainium2 kernel for the dense transformer block
(B=4, T=2048, C=1024, H=16, MLP 4x, hybrid cond/causal mask), SPMD over
8 NeuronCores, collective-free.

v2: fp8e4m3 DoubleRow matmuls for all dense GEMMs (QKV/proj/MLP/stats),
host-pretransposed fp8 weights (full-rate DMA), LN affine + biases folded
into weights/biases on the host, single-pass LN with PE-broadcast stats,
SBUF-resident x2 (no DRAM roundtrip), DVE partition-shift copies instead
of DMA in the softmax normalize chain."""
import sys
sys.path.insert(0, '/opt/trn_rl_repo')
import numpy as np
import ml_dtypes
import concourse.bass as bass
import concourse.mybir as mybir
import concourse.tile as tile
from concourse.vector_clock import ScopedClock
from concourse.bass_utils import run_bass_kernel_spmd

# ---- walrus 1-sync-wait-per-instruction workarounds ----
_installed = False


def _split_multi_waits(ordered_by_block, nc):
    for bb_name, insts in ordered_by_block.items():
        need = any(
            inst.sync_info is not None and len(inst.sync_info.on_wait) > 1
            for inst in insts
        )
        if not need:
            continue
        new_list = []
        for inst in insts:
            si = inst.sync_info
            waits = list(si.on_wait) if si is not None and si.on_wait else []
            if len(waits) > 1:
                for w in waits[:-1]:
                    nop = mybir.InstNoOp(
                        name=nc.get_next_instruction_name(),
                        ins=[],
                        outs=[],
                        bass_nofuse=True,
                    )
                    nop.engine = inst.engine
                    nop.sync_info = mybir.SyncInfo(on_wait=[w], on_update=[])
                    new_list.append(nop)
                ups = list(si.on_update) if si.on_update else []
                inst.sync_info = mybir.SyncInfo(on_wait=[waits[-1]], on_update=ups)
            new_list.append(inst)
        insts[:] = new_list


class _SplitWaitClockWait:
    def __init__(self, tc, ordered, **kw):
        import bass_rust
        self._inner = bass_rust.TileClockWait(tc, ordered, **kw)
        self._tc = tc
        self._ordered = ordered

    def __getattr__(self, a):
        return getattr(self._inner, a)

    def assign_waits(self, bb_name):
        r = self._inner.assign_waits(bb_name)
        _split_multi_waits(self._ordered, self._tc.nc)
        return r


class PatchedTileContext(tile.TileContext):
    """TileContext whose final drain carries at most one sem wait."""

    def _drain_and_barrier(self, tick_clock, wait_clock):
        probe = self.nc.sync.nop(nofuse=True)
        add = wait_clock.add_sem_waits
        add(probe.ins, ScopedClock({None: tick_clock.global_clock}))
        si = probe.ins.sync_info
        waits = list(si.on_wait) if si is not None and si.on_wait else []
        if len(waits) > 1:
            probe.ins.sync_info = mybir.SyncInfo(on_wait=[waits[0]], on_update=[])
            for w in waits[1:]:
                n = self.nc.sync.nop(nofuse=True)
                n.ins.sync_info = mybir.SyncInfo(on_wait=[w], on_update=[])
        self.nc.sync.drain()
        self.nc.all_engine_barrier()
        popped = self.nc._tile_sem_poison_stack.pop()
        assert popped is self._sem_poison
        self.nc.clear_and_free_semaphores(list(self.sems.allocated().values()))
        self.nc.all_engine_barrier()


def _install():
    global _installed
    if not _installed:
        tile.TileClockWait = _SplitWaitClockWait
        _installed = True

_install()


# ---- kernel ----

F32 = mybir.dt.float32
BF16 = mybir.dt.bfloat16
F8 = mybir.dt.float8e4
NPF8 = ml_dtypes.float8_e4m3
NPBF = ml_dtypes.bfloat16
AF = mybir.ActivationFunctionType
ALU = mybir.AluOpType
PM = mybir.MatmulPerfMode

C = 1024
T = 2048
H = 16
HD = 64
FF = 4096
COND = 256
NJ = 4
QB = 256
NCHUNK = 16
EPS = 1e-5
SCALE = 1.0 / np.sqrt(HD)

# fp8 scale constants (powers of two; data distribution is fixed)
S_H = 16.0     # LN1/LN2 output scale
S_V = 32.0     # v scale in v_ext
S_Y = 512.0    # attention-output scale
S_SQ = 4.0     # x^2 scale in LN stats
S_W = 4096.0   # Wq/Wk/Wv/Wp/W1 scale
S_W2 = 8192.0  # W2 scale

ESCALE = SCALE / (S_H * S_H * S_W * S_W)       # exp() input scale
IV_V = S_V / (S_H * S_W)                        # vps -> v_ext
IV_Y = S_Y / S_V                                # rsum -> rsumb
IV_P = 1.0 / (S_Y * S_W)                        # proj psum -> true
IV_G = 1.0 / (S_H * S_W)                        # fc1 psum -> gelu input
IV_O = 1.0 / S_W2                               # fc2 psum -> true


def kv_limit(j):
    return 4 * j + 4


def mask_slots():
    slots = []
    for j in range(NJ):
        for m in ([2, 3] if j == 0 else [4 * j, 4 * j + 1, 4 * j + 2, 4 * j + 3]):
            slots.append((j, m))
    return slots


MASK_SLOTS = mask_slots()
MASK_IDX = {s: i for i, s in enumerate(MASK_SLOTS)}
NMASK = len(MASK_SLOTS)


def build_nc(debug=False):
    nc = bass.Bass()
    xT = nc.dram_tensor("xT", [C, T], F32, kind="ExternalInput")
    xqT = nc.dram_tensor("xqT", [C, 1024], F32, kind="ExternalInput")
    wq8 = nc.dram_tensor("wq8", [128, 8, 8, 128], F8, kind="ExternalInput")
    wk8 = nc.dram_tensor("wk8", [128, 8, 8, 128], F8, kind="ExternalInput")
    wv8 = nc.dram_tensor("wv8", [128, 4, 8, 256], F8, kind="ExternalInput")
    wp8 = nc.dram_tensor("wp8", [128, 8, 8, 128], F8, kind="ExternalInput")
    w18 = nc.dram_tensor("w18", [128, 32, 8, 128], F8, kind="ExternalInput")
    w28 = nc.dram_tensor("w28", [128, 8, 32, 128], F8, kind="ExternalInput")
    bqs = nc.dram_tensor("bqs", [128, 8], F32, kind="ExternalInput")
    bps = nc.dram_tensor("bps", [128, 8], F32, kind="ExternalInput")
    b1s = nc.dram_tensor("b1s", [128, 32], F32, kind="ExternalInput")
    b2s = nc.dram_tensor("b2s", [128, 8], F32, kind="ExternalInput")
    masks = nc.dram_tensor("masks", [128, NMASK, QB], BF16, kind="ExternalInput")
    outT = nc.dram_tensor("outT", [C, 1024], F32, kind="ExternalOutput")

    dbg = {}
    if debug:
        dbg["hT"] = nc.dram_tensor("dbg_hT", [C, T], F32, kind="ExternalOutput")
        dbg["v"] = nc.dram_tensor("dbg_v", [T, H * HD], F32, kind="ExternalOutput")
        dbg["yT"] = nc.dram_tensor("dbg_yT", [C, 1024], F32, kind="ExternalOutput")
        dbg["x2T"] = nc.dram_tensor("dbg_x2T", [C, 1024], F32, kind="ExternalOutput")
        dbg["h2T"] = nc.dram_tensor("dbg_h2T", [C, 1024], F32, kind="ExternalOutput")

    from contextlib import ExitStack
    with PatchedTileContext(nc) as tc:
        with ExitStack() as stack:
            pool = lambda *a, **kw: stack.enter_context(tc.tile_pool(*a, **kw))
            big = pool(name="big", bufs=1)
            st = pool(name="st", bufs=1)
            kq_p = pool(name="kq", bufs=2)
            xin_p = pool(name="xin", bufs=4 if debug else 5)
            xb16_p = pool(name="xb16", bufs=2)
            sq8_p = pool(name="sq8", bufs=2)
            bc_p = pool(name="bc", bufs=3)
            cast_p = pool(name="cast", bufs=4)
            wsl_p = pool(name="wsl", bufs=4)
            w2sl_p = pool(name="w2sl", bufs=2)
            wv8_p = pool(name="wv8p", bufs=4)
            se_p = pool(name="se", bufs=4)
            rows_p = pool(name="rows", bufs=2)
            sml_p = pool(name="sml", bufs=2)
            out_p = pool(name="outp", bufs=1 if debug else 2)
            psb = pool(name="psb", bufs=2, space="PSUM")
            pss = pool(name="pss", bufs=2, space="PSUM")
            psy = pool(name="psy", bufs=2, space="PSUM")
            hT8 = big.tile([128, 8, T], F8, tag="A")
            hqT8 = big.tile([128, 8, 1024], F8, tag="E")
            yT8 = big.tile([128, 8, 1024], F8, tag="Y")
            v_ext = big.tile([128, NCHUNK, H, HD + 1], F8, tag="B")
            x232 = big.tile([128, 8, 1024], F32, tag="X2")
            mask_sb = st.tile([128, NMASK, QB], BF16)

            ones_mu = st.tile([128, 1], BF16)
            ivy64 = st.tile([1, 64], BF16)
            ones_bc = st.tile([1, 128], BF16)

            ones8 = st.tile([128, 2, 64], F8)
            eps1 = st.tile([1, 1], F32)
            nc.vector.memset(ones_mu, 1.0)
            nc.vector.memset(ivy64, IV_Y)
            nc.vector.memset(ones_bc, 1.0)

            nc.vector.memset(ones8, 1.0)
            nc.vector.memset(eps1, EPS)

            bqs_sb = st.tile([128, 8], F32)
            bps_sb = st.tile([128, 8], F32)
            b1s_sb = st.tile([128, 32], F32)
            b2s_sb = st.tile([128, 8], F32)
            nc.sync.dma_start(out=bqs_sb, in_=bqs[:, :])
            nc.sync.dma_start(out=bps_sb, in_=bps[:, :])
            nc.sync.dma_start(out=b1s_sb, in_=b1s[:, :])
            nc.sync.dma_start(out=b2s_sb, in_=b2s[:, :])
            nc.sync.dma_start(out=mask_sb, in_=masks[:, :, :])

            # ---------- LayerNorm core (x - mu) * rstd -> fp8*S_H ----------
            # src(t, cc) -> f32 AP [128, 512]; out_tile [128, 8, ntok]
            def ln_T(src, ntok, out_tile):
                nt = ntok // 512
                for t in range(nt):
                    xb16 = xb16_p.tile([128, 8, 512], BF16, tag="xb")
                    sq8t = sq8_p.tile([128, 8, 512], F8, tag="sq")
                    mu_ps = pss.tile([1, 512], F32, tag="pstd")
                    sq_ps = pss.tile([64, 512], F32, tag="pstd")
                    for cc in range(8):
                        xt = src(t, cc)
                        nc.scalar.activation(
                            out=xb16[:, cc, :], in_=xt, func=AF.Identity)
                        nc.gpsimd.tensor_tensor(
                            out=sq8t[:, cc, :], in0=xt, in1=xt, op=ALU.mult)
                        nc.tensor.matmul(
                            mu_ps, ones_mu, xb16[:, cc, :],
                            start=(cc == 0), stop=(cc == 7))
                    for pr in range(4):
                        nc.tensor.matmul(
                            sq_ps, ones8, sq8t[:, 2 * pr:2 * pr + 2, :],
                            start=(pr == 0), stop=(pr == 3),
                            perf_mode=PM.DoubleRow)
                    mu = rows_p.tile([1, 512], F32, tag="mu")
                    ex2 = rows_p.tile([1, 512], F32, tag="ex2")
                    nc.vector.tensor_scalar_mul(mu, mu_ps, 1.0 / C)
                    nc.vector.tensor_scalar_mul(ex2, sq_ps[0:1, :], 1.0 / C)
                    var = rows_p.tile([1, 512], F32, tag="var")
                    nc.vector.scalar_tensor_tensor(
                        out=var, in0=mu, scalar=-1.0, in1=mu,
                        op0=ALU.mult, op1=ALU.mult)
                    nc.vector.tensor_add(var, var, ex2)
                    lnv = rows_p.tile([1, 512], F32, tag="lnv")
                    nc.scalar.activation(out=lnv, in_=var, func=AF.Ln, bias=eps1)
                    rstd = rows_p.tile([1, 512], F32, tag="rstd")
                    nc.scalar.activation(out=rstd, in_=lnv, func=AF.Exp, scale=-0.5)
                    rstd16 = rows_p.tile([1, 512], BF16, tag="rstd16")
                    nc.vector.tensor_copy(out=rstd16, in_=rstd)
                    bb16 = rows_p.tile([1, 512], BF16, tag="bb16")
                    nc.vector.scalar_tensor_tensor(
                        out=bb16, in0=mu, scalar=-S_H, in1=rstd,
                        op0=ALU.mult, op1=ALU.mult)
                    a_ps = pss.tile([128, 512], F32, tag="pstd")
                    nc.tensor.matmul(a_ps, ones_bc, rstd16, start=True, stop=True)
                    a16 = bc_p.tile([128, 512], BF16, tag="a16")
                    nc.vector.tensor_copy(out=a16, in_=a_ps)
                    b_ps = pss.tile([128, 512], F32, tag="pstd")
                    nc.tensor.matmul(b_ps, ones_bc, bb16, start=True, stop=True)
                    b16 = bc_p.tile([128, 512], BF16, tag="b16")
                    nc.vector.tensor_copy(out=b16, in_=b_ps)
                    for cc in range(8):
                        t1 = cast_p.tile([128, 512], BF16, tag="t1")
                        nc.vector.scalar_tensor_tensor(
                            out=t1, in0=xb16[:, cc, :], scalar=S_H, in1=a16,
                            op0=ALU.mult, op1=ALU.mult)
                        eng = nc.vector if cc % 2 == 0 else nc.gpsimd
                        eng.tensor_tensor(
                            out=out_tile[:, cc, 512 * t:512 * (t + 1)],
                            in0=b16, in1=t1, op=ALU.add)

            def src_xT(t, cc):
                xt = xin_p.tile([128, 512], F32, tag="x")
                nc.sync.dma_start(
                    out=xt, in_=xT[128 * cc:128 * (cc + 1), 512 * t:512 * (t + 1)])
                return xt

            def src_xqT(t, cc):
                xt = xin_p.tile([128, 512], F32, tag="x")
                nc.sync.dma_start(
                    out=xt, in_=xqT[128 * cc:128 * (cc + 1), 512 * t:512 * (t + 1)])
                return xt

            def src_x2(t, cc):
                return x232[:, cc, 512 * t:512 * (t + 1)]

            ln_T(src_xT, T, hT8)

            if debug:
                for cc in range(8):
                    for t in range(4):
                        dt_ = out_p.tile([128, 512], F32, tag="dbg")
                        nc.vector.tensor_scalar_mul(
                            dt_, hT8[:, cc, 512 * t:512 * (t + 1)], 1.0 / S_H)
                        nc.sync.dma_start(
                            out=dbg["hT"][128 * cc:128 * (cc + 1), 512 * t:512 * (t + 1)],
                            in_=dt_)

            # ---------- V ----------
            wv_sls = []
            for fh in range(4):
                wv_sl = wv8_p.tile([128, 8, 256], F8, tag="wv")
                nc.sync.dma_start(out=wv_sl, in_=wv8[:, fh, :, :])
                wv_sls.append(wv_sl)
            for m in range(NCHUNK):
                for fh in range(4):
                    vps = pss.tile([128, 256], F32, tag="pstd")
                    for pr in range(4):
                        nc.tensor.matmul(
                            vps,
                            hT8[:, 2 * pr:2 * pr + 2, 128 * m:128 * (m + 1)],
                            wv_sls[fh][:, 2 * pr:2 * pr + 2, :],
                            start=(pr == 0), stop=(pr == 3),
                            perf_mode=PM.DoubleRow)
                    nc.vector.tensor_scalar_mul(
                        v_ext[:, m, 4 * fh:4 * (fh + 1), 0:HD],
                        vps.rearrange("p (h d) -> p h d", h=4), IV_V)
                nc.vector.memset(v_ext[:, m, :, HD:HD + 1], 1.0)

            if debug:
                for m in range(NCHUNK):
                    for hf in range(2):
                        dt_ = out_p.tile([128, 512], F32, tag="dbg")
                        nc.vector.tensor_scalar_mul(
                            dt_.rearrange("p (h d) -> p h d", h=8),
                            v_ext[:, m, 8 * hf:8 * (hf + 1), 0:HD], 1.0 / S_V)
                        nc.sync.dma_start(
                            out=dbg["v"][128 * m:128 * (m + 1), 512 * hf:512 * (hf + 1)],
                            in_=dt_)

            # ---------- LN1 on query columns ----------
            ln_T(src_xqT, 1024, hqT8)

            # ---------- attention ----------
            for p in range(8):
                wk_sl = wsl_p.tile([128, 8, 128], F8, tag="wk")
                nc.sync.dma_start(out=wk_sl, in_=wk8[:, p, :, :])
                kpair = kq_p.tile([128, T], BF16, tag="KP")
                for t in range(4):
                    kps = pss.tile([128, 512], F32, tag="pstd")
                    for pr in range(4):
                        nc.tensor.matmul(
                            kps, wk_sl[:, 2 * pr:2 * pr + 2, :],
                            hT8[:, 2 * pr:2 * pr + 2, 512 * t:512 * (t + 1)],
                            start=(pr == 0), stop=(pr == 3),
                            perf_mode=PM.DoubleRow)
                    nc.vector.tensor_copy(
                        out=kpair[:, 512 * t:512 * (t + 1)], in_=kps)

                wq_sl = wsl_p.tile([128, 8, 128], F8, tag="wk")
                nc.sync.dma_start(out=wq_sl, in_=wq8[:, p, :, :])
                qpair = kq_p.tile([128, 1024], BF16, tag="QP")
                for t in range(2):
                    qps = pss.tile([128, 512], F32, tag="pstd")
                    for pr in range(4):
                        nc.tensor.matmul(
                            qps, wq_sl[:, 2 * pr:2 * pr + 2, :],
                            hqT8[:, 2 * pr:2 * pr + 2, 512 * t:512 * (t + 1)],
                            start=(pr == 0), stop=(pr == 3),
                            perf_mode=PM.DoubleRow)
                    nc.scalar.activation(
                        out=qpair[:, 512 * t:512 * (t + 1)], in_=qps,
                        func=AF.Identity, bias=bqs_sb[:, p:p + 1])

                for odd in range(2):
                    h = 2 * p + odd
                    ko = 64 * odd
                    for j in range(NJ):
                        L = kv_limit(j)
                        qs = slice(QB * j, QB * (j + 1))
                        yps = psy.tile([65, QB], F32, tag="yps")
                        for g0 in range(0, L, 4):
                            gl = min(4, L - g0)
                            sps = psb.tile([128, 4, QB], F32, tag="pbig")
                            for mi in range(gl):
                                m = g0 + mi
                                nc.tensor.matmul(
                                    sps[:, mi, :],
                                    kpair[ko:ko + 64, 128 * m:128 * (m + 1)],
                                    qpair[ko:ko + 64, qs],
                                    start=True, stop=True)
                            se_t = se_p.tile([128, 4, QB], BF16, tag="se")
                            nc.scalar.activation(
                                out=se_t[:, 0:gl, :], in_=sps[:, 0:gl, :],
                                func=AF.Exp, scale=float(ESCALE))
                            for mi in range(gl):
                                m = g0 + mi
                                if (j, m) in MASK_IDX:
                                    kmi = MASK_IDX[(j, m)]
                                    meng = nc.vector if kmi % 2 == 0 else nc.gpsimd
                                    meng.tensor_tensor(
                                        out=se_t[:, mi, :], in0=se_t[:, mi, :],
                                        in1=mask_sb[:, kmi, :], op=ALU.mult)
                            for mi in range(gl):
                                m = g0 + mi
                                nc.tensor.matmul(
                                    yps, v_ext[:, m, h, :], se_t[:, mi, :],
                                    start=(m == 0), stop=(m == L - 1))
                        rsumb = sml_p.tile([1, QB], BF16, tag="rsumb")
                        with nc.allow_low_precision(reason="rsum feeds fp8 y"):
                            nc.vector.reciprocal(out=rsumb, in_=yps[64:65, :])
                        rbps = psy.tile([64, QB], F32, tag="yps")
                        nc.tensor.matmul(rbps, ivy64, rsumb, start=True, stop=True)
                        rb = sml_p.tile([64, QB], BF16, tag="rb")
                        nc.vector.tensor_copy(out=rb, in_=rbps)
                        nc.vector.tensor_mul(
                            yT8[ko:ko + 64, p, qs], yps[0:64, :], rb)

            if debug:
                for p in range(8):
                    for t in range(2):
                        dt_ = out_p.tile([128, 512], F32, tag="dbg")
                        nc.vector.tensor_scalar_mul(
                            dt_, yT8[:, p, 512 * t:512 * (t + 1)], 1.0 / S_Y)
                        nc.sync.dma_start(
                            out=dbg["yT"][128 * p:128 * (p + 1), 512 * t:512 * (t + 1)],
                            in_=dt_)

            # ---------- proj + residual -> x232 (SBUF) ----------
            for co in range(8):
                wp_sl = wsl_p.tile([128, 8, 128], F8, tag="wk")
                nc.sync.dma_start(out=wp_sl, in_=wp8[:, co, :, :])
                for t in range(2):
                    pps = pss.tile([128, 512], F32, tag="pstd")
                    for pr in range(4):
                        nc.tensor.matmul(
                            pps, wp_sl[:, 2 * pr:2 * pr + 2, :],
                            yT8[:, 2 * pr:2 * pr + 2, 512 * t:512 * (t + 1)],
                            start=(pr == 0), stop=(pr == 3),
                            perf_mode=PM.DoubleRow)
                    p16 = cast_p.tile([128, 512], BF16, tag="p16")
                    nc.vector.tensor_scalar(
                        out=p16, in0=pps, scalar1=IV_P, scalar2=bps_sb[:, co:co + 1],
                        op0=ALU.mult, op1=ALU.add)
                    xt = xin_p.tile([128, 512], F32, tag="x")
                    nc.sync.dma_start(
                        out=xt, in_=xqT[128 * co:128 * (co + 1), 512 * t:512 * (t + 1)])
                    nc.vector.tensor_add(
                        x232[:, co, 512 * t:512 * (t + 1)], p16, xt)
                    if debug:
                        dt_ = out_p.tile([128, 512], F32, tag="dbg")
                        nc.vector.tensor_copy(
                            out=dt_, in_=x232[:, co, 512 * t:512 * (t + 1)])
                        nc.sync.dma_start(
                            out=dbg["x2T"][128 * co:128 * (co + 1), 512 * t:512 * (t + 1)],
                            in_=dt_)

            # ---------- LN2 ----------
            h2T8 = big.tile([128, 8, 1024], F8, tag="E")  # reuses hqT8 slot
            ln_T(src_x2, 1024, h2T8)

            if debug:
                for cc in range(8):
                    for t in range(2):
                        dt_ = out_p.tile([128, 512], F32, tag="dbg")
                        nc.vector.tensor_scalar_mul(
                            dt_, h2T8[:, cc, 512 * t:512 * (t + 1)], 1.0 / S_H)
                        nc.sync.dma_start(
                            out=dbg["h2T"][128 * cc:128 * (cc + 1), 512 * t:512 * (t + 1)],
                            in_=dt_)

            # ---------- MLP ----------
            gT8 = big.tile([128, 32, 512], F8, tag="A")  # reuses hT8 slot
            for t in range(2):
                rs = slice(512 * t, 512 * (t + 1))
                for f in range(32):
                    w1_sl = wsl_p.tile([128, 8, 128], F8, tag="wk")
                    nc.sync.dma_start(out=w1_sl, in_=w18[:, f, :, :])
                    fps = pss.tile([128, 512], F32, tag="pstd")
                    for pr in range(4):
                        nc.tensor.matmul(
                            fps, w1_sl[:, 2 * pr:2 * pr + 2, :],
                            h2T8[:, 2 * pr:2 * pr + 2, rs],
                            start=(pr == 0), stop=(pr == 3),
                            perf_mode=PM.DoubleRow)
                    nc.scalar.activation(
                        out=gT8[:, f, :], in_=fps, func=AF.Gelu,
                        bias=b1s_sb[:, f:f + 1], scale=IV_G)
                for co in range(8):
                    w2_sl = w2sl_p.tile([128, 32, 128], F8, tag="w2")
                    nc.sync.dma_start(out=w2_sl, in_=w28[:, co, :, :])
                    ops_ = pss.tile([128, 512], F32, tag="pstd")
                    for pr in range(16):
                        nc.tensor.matmul(
                            ops_, w2_sl[:, 2 * pr:2 * pr + 2, :],
                            gT8[:, 2 * pr:2 * pr + 2, :],
                            start=(pr == 0), stop=(pr == 15),
                            perf_mode=PM.DoubleRow)
                    o16 = cast_p.tile([128, 512], BF16, tag="p16")
                    nc.vector.tensor_scalar(
                        out=o16, in0=ops_, scalar1=IV_O, scalar2=b2s_sb[:, co:co + 1],
                        op0=ALU.mult, op1=ALU.add)
                    o32 = out_p.tile([128, 512], F32, tag="o32")
                    nc.gpsimd.tensor_tensor(
                        out=o32, in0=o16, in1=x232[:, co, rs], op=ALU.add)
                    nc.sync.dma_start(
                        out=outT[128 * co:128 * (co + 1), rs], in_=o32)

    return nc


# ===================== host side =====================

def make_masks(rho):
    out = np.zeros((NMASK, 128, QB), dtype=np.float32)
    for i, (j, m) in enumerate(MASK_SLOTS):
        q0 = 512 * j + QB * rho
        q = q0 + np.arange(QB)[None, :]
        k = 128 * m + np.arange(128)[:, None]
        allowed = (k < COND) | ((q >= COND) & (k >= COND) & (k <= q))
        out[i] = allowed
    return np.ascontiguousarray(out.transpose(1, 0, 2)).astype(NPBF)


def _wlayout(w, nf):
    """[C_in, F_out] -> [128, F_out//nf, C_in//128, nf] (p, blk, cc, f)."""
    ci, fo = w.shape
    return np.ascontiguousarray(
        w.reshape(ci // 128, 128, fo // nf, nf).transpose(1, 2, 0, 3))


def _w2layout(w):
    """W2 [FF, C] -> [128, 8, 32, 128] (p, co, fc, f)."""
    return np.ascontiguousarray(
        w.reshape(32, 128, 8, 128).transpose(1, 2, 0, 3))


def _q8(w, s):
    return (np.asarray(w, np.float32) * s).astype(NPF8)


def _btile(v, k):
    """[N] f32 -> [128, N//128] with column i = slice i."""
    return np.ascontiguousarray(np.asarray(v, np.float32).reshape(k, 128).T)


def shard_inputs(inputs):
    x = np.asarray(inputs["x"], np.float32)
    g1 = np.asarray(inputs["ln1_g"], np.float32)
    bt1 = np.asarray(inputs["ln1_b"], np.float32)
    g2 = np.asarray(inputs["ln2_g"], np.float32)
    bt2 = np.asarray(inputs["ln2_b"], np.float32)
    Wq = np.asarray(inputs["Wq"], np.float32)
    Wk = np.asarray(inputs["Wk"], np.float32)
    Wv = np.asarray(inputs["Wv"], np.float32)
    Wp = np.asarray(inputs["Wp"], np.float32)
    W1 = np.asarray(inputs["W1"], np.float32)
    W2 = np.asarray(inputs["W2"], np.float32)

    Wq_eff = g1[:, None] * Wq
    Wk_eff = g1[:, None] * Wk
    Wv_eff = g1[:, None] * Wv
    W1_eff = g2[:, None] * W1

    bq_eff = np.asarray(inputs["bq"], np.float32) + bt1 @ Wq
    bv_eff = np.asarray(inputs["bv"], np.float32) + bt1 @ Wv
    bp_eff = np.asarray(inputs["bp"], np.float32) + bv_eff @ Wp
    b1_eff = np.asarray(inputs["b1"], np.float32) + bt2 @ W1
    b2_eff = np.asarray(inputs["b2"], np.float32)

    ins_common = {
        "wq8": _q8(_wlayout(Wq_eff, 128), S_W),
        "wk8": _q8(_wlayout(Wk_eff, 128), S_W),
        "wv8": _q8(_wlayout(Wv_eff, 256), S_W),
        "wp8": _q8(_wlayout(Wp, 128), S_W),
        "w18": _q8(_wlayout(W1_eff, 128), S_W),
        "w28": _q8(_w2layout(W2), S_W2),
        "bqs": _btile(bq_eff * (S_H * S_W), 8),
        "bps": _btile(bp_eff, 8),
        "b1s": _btile(b1_eff, 32),
        "b2s": _btile(b2_eff, 8),
    }
    in_maps, row_sets = [], []
    for c in range(8):
        b, rho = c // 2, c % 2
        rows = np.concatenate(
            [np.arange(512 * j + QB * rho, 512 * j + QB * rho + QB) for j in range(4)])
        row_sets.append((b, rows))
        m = dict(ins_common)
        m["xT"] = np.ascontiguousarray(x[b].T)
        m["xqT"] = np.ascontiguousarray(x[b][rows].T)
        m["masks"] = make_masks(rho)
        in_maps.append(m)
    return in_maps, row_sets


_cached_nc = {}


def get_nc(debug=False):
    if debug not in _cached_nc:
        _cached_nc[debug] = build_nc(debug=debug)
    return _cached_nc[debug]


def run(inputs, debug=False):
    nc = get_nc(debug=debug)
    in_maps, row_sets = shard_inputs(inputs)
    res = run_bass_kernel_spmd(nc, in_maps, core_ids=list(range(8)))
    x = np.asarray(inputs["x"], np.float32)
    out = np.empty_like(x)
    for c in range(8):
        b, rows = row_sets[c]
        out[b][rows] = res.results[c]["outT"].T
    return out, res, row_sets


def kernel(**inputs):
    out, _, _ = run(inputs, debug=False)
    return out
